# revision 24
# baseline (speedup 1.0000x reference)
"""Trainium2 Bass kernel for a 4-head GAT layer (N=4096, D=256, O=64, H=4).

Math (reference):
    feat[h] = X @ W[h]                                  [N, O]
    s[h,i] = feat[h,i] @ a_src[h],  t[h,j] = feat[h,j] @ a_dst[h]
    score[h,i,j] = leaky_relu(s_i + t_j, 0.2), masked by A>0, softmax over j
    out[i, h*O+o] = sum_j attn[h,i,j] feat[h,j,o] + b[h,o]

Key factorization used on-device (avoids 67M-element exp/leaky passes):
    exp(leaky_relu(x)) = max(e^x, e^{0.2x}); with x = s_i + t_j both branches
    factor.  With M2 = A * [x >= 0] and M1 = A - M2:
      numer = e^{0.2 s} * [ (A@(q*f) - M2@(q*f)) + e^{0.8 s} * (M2@(v*f)) ]
    where v = e^t, q = e^{0.2 t}; the common e^{0.2 s} cancels in the softmax
    ratio.  Row sums come from an appended ones-column in the rhs panels.

    Masks are built with DVE fast modes: tensor_scalar is_le (4x mode) for
    m = [-s_i <= t_j] batched [128 x 512], then tensor_tensor mult by the A
    tile (2x mode).  The A-branch accumulates in a prepass fused into head
    pass 0; four single-head passes keep PSUM at 4+4 banks.

Sharding: destination rows are split 512/core across 8 cores; source-side
features (all N) are recomputed per core (cheap).  No collectives.
b is always zero in setup_inputs but is added on the host anyway.
"""

from contextlib import ExitStack

import numpy as np

import concourse.bass as bass
import concourse.tile as tile
import concourse.mybir as mybir
from concourse import bacc
from concourse.bass_utils import run_bass_kernel_spmd

P = 128
IN_DIM = 256
OUT_DIM = 64
HEADS = 4
N_TOTAL = 4096
N_CORES = 8
ROWS = N_TOTAL // N_CORES  # 512

F32 = mybir.dt.float32
F16 = mybir.dt.float16

AL = mybir.AluOpType
AF = mybir.ActivationFunctionType

GRP = 66          # [feat(64) | 1 | 1] per head in the fe panel (even => 4B aligned)
FET_C = 4 * GRP + 4   # 268 cols per n-tile in fet
VQ_C = 2 * GRP    # 132 cols per head in the vfqf panel
PAN_C = 4 * VQ_C  # 528 cols per n-tile in vfqf


def build_program(n_total=N_TOTAL, rows=ROWS, num_devices=N_CORES):
    """Build the per-core SPMD program (same program on all cores; per-core
    data arrives via the input map)."""
    ntiles = n_total // P   # source-node tiles (j)
    nib = rows // P         # destination row blocks per core
    njt = ntiles

    nc = bacc.Bacc("TRN2", target_bir_lowering=False, debug=False,
                   num_devices=num_devices)

    XT = nc.dram_tensor("XT", [IN_DIM, n_total], F16, kind="ExternalInput")
    XTOWN = nc.dram_tensor("XTOWN", [IN_DIM, rows], F16, kind="ExternalInput")
    W8 = nc.dram_tensor("W8", [IN_DIM, 260], F16, kind="ExternalInput")
    W4 = nc.dram_tensor("W4", [IN_DIM, 4], F16, kind="ExternalInput")
    WSRCB = nc.dram_tensor("WSRCB", [IN_DIM, 4 * P], F16, kind="ExternalInput")
    AT = nc.dram_tensor("AT", [n_total, rows], F16, kind="ExternalInput")
    OUT = nc.dram_tensor("OUT", [rows, HEADS * OUT_DIM], F32,
                         kind="ExternalOutput")

    with tile.TileContext(nc) as tc, ExitStack() as ctx:
        big = ctx.enter_context(tc.tile_pool(name="big", bufs=1))

        # ---- Phase 0: load everything ----
        # Small tensors first: the sbc / s_own / feat matmuls need them, and
        # DMA queues drain FIFO -- putting the 6MB of XT/AT ahead of them
        # stalls all compute behind ~25us of bulk DMA.
        xtown_sb = big.tile([P, 2 * rows], F16, tag="xtown")
        for d in range(2):
            nc.sync.dma_start(xtown_sb[:, d * rows:(d + 1) * rows],
                              XTOWN[d * P:(d + 1) * P, :])
        w8_sb = big.tile([P, 2 * 260], F16, tag="w8")
        for d in range(2):
            nc.sync.dma_start(w8_sb[:, d * 260:(d + 1) * 260],
                              W8[d * P:(d + 1) * P, :])
        w4_sb = big.tile([P, 2 * 4], F16, tag="w4")
        for d in range(2):
            nc.sync.dma_start(w4_sb[:, d * 4:(d + 1) * 4],
                              W4[d * P:(d + 1) * P, :])
        wsrcb_sb = big.tile([P, 2 * 4 * P], F16, tag="wsrcb")
        for d in range(2):
            nc.sync.dma_start(wsrcb_sb[:, d * 4 * P:(d + 1) * 4 * P],
                              WSRCB[d * P:(d + 1) * P, :])
        xt_sb = big.tile([P, 2 * n_total], F16, tag="xt")
        nch = 8
        for c in range(nch):
            w = n_total // nch
            for d in range(2):
                nc.sync.dma_start(
                    xt_sb[:, d * n_total + c * w: d * n_total + (c + 1) * w],
                    XT[d * P:(d + 1) * P, c * w:(c + 1) * w])
        at_sb = big.tile([P, njt * rows], F16, tag="at")
        for jt in range(njt):
            nc.sync.dma_start(at_sb[:, jt * rows:(jt + 1) * rows],
                              AT[jt * P:(jt + 1) * P, :])

        # ---- Phase 1: feat + t matmuls; vf/qf panels via ACT scaled copies ----
        # panel layout per n-tile, per head (132 cols):
        #   [vf(64) | v | pad | qf(64) | q | pad]
        t16 = big.tile([P, ntiles * 4], F16, tag="t16")
        t3 = t16[:].rearrange("p (n c) -> p n c", c=4)
        vq = big.tile([P, ntiles * 8], F32, tag="vq")
        vq3 = vq[:].rearrange("p (n c) -> p n c", c=8)
        tpos = big.tile([P, ntiles * 4], F32, tag="tpos")
        tposk = big.tile([P, ntiles * 4], F32, tag="tposk")
        fe = big.tile([P, ntiles * 4 * GRP], F16, tag="fe")
        fe3 = fe[:].rearrange("p (n c) -> p n c", c=4 * GRP)
        vfqf = big.tile([P, ntiles * PAN_C], F16, tag="vfqf")

        # s broadcast rows (NEGATED: host supplies -w_src), grouped per head:
        # sbc[:, h*rows + i] = -s_src[h, i]
        sbc = big.tile([P, 4 * rows], F16, tag="sbc")
        with tc.tile_pool(name="psb", bufs=2, space=bass.MemorySpace.PSUM) as psb:
            for ib in range(nib):
                ps = psb.tile([P, 4 * P], F32, tag="ps_sb")
                for h in range(HEADS):
                    for d in range(2):
                        nc.tensor.matmul(
                            ps[:, h * P:(h + 1) * P],
                            wsrcb_sb[:, d * 4 * P + h * P: d * 4 * P + (h + 1) * P],
                            xtown_sb[:, d * rows + ib * P: d * rows + (ib + 1) * P],
                            start=(d == 0), stop=(d == 1))
                for h in range(HEADS):
                    nc.vector.tensor_copy(
                        sbc[:, h * rows + ib * P: h * rows + (ib + 1) * P],
                        ps[:, h * P:(h + 1) * P])

        s_own = big.tile([P, nib * 4], F32, tag="s_own")
        w_cat = big.tile([P, nib * 4], F32, tag="w_cat")
        with tc.tile_pool(name="pso", bufs=1, space=bass.MemorySpace.PSUM) as pso:
            ps = pso.tile([P, nib * 4], F32, tag="ps_so")
            for ib in range(nib):
                for d in range(2):
                    nc.tensor.matmul(
                        ps[:, ib * 4:(ib + 1) * 4],
                        xtown_sb[:, d * rows + ib * P: d * rows + (ib + 1) * P],
                        w4_sb[:, d * 4:(d + 1) * 4],
                        start=(d == 0), stop=(d == 1))
            nc.vector.tensor_copy(s_own[:], ps[:])
        nc.scalar.activation(w_cat[:], s_own[:], AF.Exp, scale=0.8)

        # ---- feat + t matmuls; panels via DVE 4x tensor_scalar ----
        t16 = big.tile([P, ntiles * 4], F16, tag="t16")
        t3 = t16[:].rearrange("p (n c) -> p n c", c=4)
        vq = big.tile([P, ntiles * 8], F32, tag="vq")
        vq3 = vq[:].rearrange("p (n c) -> p n c", c=8)
        tpos = big.tile([P, ntiles * 4], F32, tag="tpos")
        tposk = big.tile([P, ntiles * 4], F32, tag="tposk")
        fe = big.tile([P, ntiles * 4 * GRP], F16, tag="fe")
        fe3 = fe[:].rearrange("p (n c) -> p n c", c=4 * GRP)
        vfqf = big.tile([P, ntiles * PAN_C], F16, tag="vfqf")

        CHUNK = min(4, ntiles)  # n-tiles per exp chunk
        with tc.tile_pool(name="pfeat", bufs=6, space=bass.MemorySpace.PSUM) as pf:
            for nt0 in range(0, ntiles, CHUNK):
                pss = []
                for nt in range(nt0, nt0 + CHUNK):
                    ps = pf.tile([P, 264], F32, tag="ps")
                    pss.append(ps)
                    for d in range(2):
                        nc.tensor.matmul(
                            ps[:, 0:260],
                            xt_sb[:, d * n_total + nt * P: d * n_total + (nt + 1) * P],
                            w8_sb[:, d * 260:(d + 1) * 260],
                            start=(d == 0), stop=(d == 1))
                    nc.scalar.activation(t3[:, nt, :], ps[:, 256:260], AF.Copy)
                ch = slice(nt0, nt0 + CHUNK)
                nc.scalar.activation(vq3[:, ch, 0:4], t3[:, ch, :], AF.Exp)
                nc.scalar.activation(vq3[:, ch, 4:8], t3[:, ch, :], AF.Exp, scale=0.2)
                nc.vector.tensor_copy(
                    tpos[:, nt0 * 4:(nt0 + CHUNK) * 4], t3[:, ch, :])
                nc.vector.tensor_scalar_mul(
                    tposk[:, nt0 * 4:(nt0 + CHUNK) * 4], t3[:, ch, :], 1.0e4)
                for nt in range(nt0, nt0 + CHUNK):
                    ps = pss[nt - nt0]
                    fe_g = fe3[:, nt, :].rearrange("p (g c) -> p g c", c=GRP)
                    nc.scalar.activation(
                        fe_g[:, :, 0:64],
                        ps[:, 0:256].rearrange("p (g c) -> p g c", c=64),
                        AF.Copy)
                    nc.vector.memset(fe_g[:, :, 64:66], 1.0)
                    pan = vfqf[:, nt * PAN_C:(nt + 1) * PAN_C]
                    for h in range(2):
                        nc.vector.tensor_scalar_mul(
                            pan[:, h * VQ_C: h * VQ_C + GRP],
                            fe_g[:, h, :], vq3[:, nt, h:h + 1])
                        nc.vector.tensor_scalar_mul(
                            pan[:, h * VQ_C + GRP: (h + 1) * VQ_C],
                            fe_g[:, h, :], vq3[:, nt, 4 + h:5 + h])
                    # heads 2-3 panels on the Scalar engine (slack window)
                    for h in range(2, HEADS):
                        nc.scalar.activation(
                            pan[:, h * VQ_C: h * VQ_C + GRP],
                            fe_g[:, h, :], AF.Copy, scale=vq3[:, nt, h:h + 1])
                        nc.scalar.activation(
                            pan[:, h * VQ_C + GRP: (h + 1) * VQ_C],
                            fe_g[:, h, :], AF.Copy, scale=vq3[:, nt, 4 + h:5 + h])

        # ---- Phase 4: fused A-prepass (inside pass 0) + 4 single-head passes ----
        m_pool = ctx.enter_context(tc.tile_pool(name="m", bufs=6))
        m2_pool = ctx.enter_context(tc.tile_pool(name="m2", bufs=8))
        out_sb_pool = ctx.enter_context(tc.tile_pool(name="osb", bufs=4))
        e_pool = ctx.enter_context(tc.tile_pool(name="epi", bufs=6))
        ca_all = []
        for ib in range(nib):
            ca_ib = big.tile([P, 260], F32, tag=f"ca{ib}")
            ca_all.append(ca_ib)
        out_sbs = []
        for ib in range(nib):
            osb = out_sb_pool.tile([P, HEADS * OUT_DIM], F32, tag="outsb")
            out_sbs.append(osb)
        with tc.tile_pool(name="pA", bufs=4, space=bass.MemorySpace.PSUM) as pA, \
             tc.tile_pool(name="pB", bufs=4, space=bass.MemorySpace.PSUM) as pB:
            pa = []
            for ib in range(nib):
                pa_ib = pA.tile([P, 260], F32, tag="pa")
                pa.append(pa_ib)
            for h in range(HEADS):
                pb = []
                for ib in range(nib):
                    pb_ib = pB.tile([P, 130], F32, tag="pb")
                    pb.append(pb_ib)
                for jt in range(njt):
                    a_row = at_sb[:, jt * rows:(jt + 1) * rows]
                    pan = vfqf[:, jt * PAN_C:(jt + 1) * PAN_C]
                    if h == 0:
                        # A-branch: all four heads' qf panels at once
                        qf_all = pan[:].rearrange(
                            "p (g c) -> p g c", c=VQ_C)[:, :, GRP:GRP + 65]
                        for ib in range(nib):
                            nc.tensor.matmul(
                                pa[ib][:],
                                at_sb[:, jt * rows + ib * P: jt * rows + (ib + 1) * P],
                                qf_all,
                                start=(jt == 0), stop=(jt == njt - 1))
                    mb = m_pool.tile([P, rows], F16, tag="mb")
                    if h >= 2:
                        # step via saturated sigmoid on the (idle) Scalar
                        # engine: sigmoid(1e4*(s_i + t_j)); sbc holds -s
                        nc.scalar.activation(
                            mb[:], sbc[:, h * rows:(h + 1) * rows],
                            AF.Sigmoid, scale=-1.0e4,
                            bias=tposk[:, jt * 4 + h: jt * 4 + h + 1])
                    else:
                        # m = [s_i + t_j >= 0] = [-s_i <= t_j]
                        nc.vector.tensor_scalar(
                            mb[:], sbc[:, h * rows:(h + 1) * rows],
                            tpos[:, jt * 4 + h: jt * 4 + h + 1],
                            None, AL.is_le)
                    m2 = m2_pool.tile([P, rows], F16, tag="m2b")
                    nc.vector.tensor_tensor(m2[:], mb[:], a_row, AL.mult)
                    vfqf_h = pan[:, h * VQ_C: h * VQ_C + 2 * GRP].rearrange(
                        "p (g c) -> p g c", c=GRP)[:, :, 0:65]
                    for ib in range(nib):
                        nc.tensor.matmul(
                            pb[ib][:],
                            m2[:, ib * P:(ib + 1) * P],
                            vfqf_h,
                            start=(jt == 0), stop=(jt == njt - 1))
                if h == 0:
                    for ib in range(nib):
                        nc.scalar.activation(ca_all[ib][:], pa[ib][:], AF.Copy)
                # epilogue for head h
                for ib in range(nib):
                    dh = e_pool.tile([P, 65], F32, tag="dh")
                    nc.vector.tensor_sub(
                        dh[:], ca_all[ib][:, h * 65:(h + 1) * 65],
                        pb[ib][:, 65:130])
                    zh = e_pool.tile([P, 65], F32, tag="zh")
                    nc.vector.scalar_tensor_tensor(
                        zh[:], pb[ib][:, 0:65],
                        w_cat[:, ib * 4 + h: ib * 4 + h + 1],
                        dh[:], AL.mult, AL.add)
                    rc = e_pool.tile([P, 1], F32, tag="rc")
                    nc.vector.reciprocal(rc[:], zh[:, 64:65])
                    nc.vector.tensor_scalar_mul(
                        out_sbs[ib][:, h * OUT_DIM:(h + 1) * OUT_DIM],
                        zh[:, 0:OUT_DIM], rc[:])
        for ib in range(nib):
            nc.sync.dma_start(OUT[ib * P:(ib + 1) * P, :], out_sbs[ib][:])

    nc.compile()
    return nc


def prep_inputs(X, A, W, a, n_total=N_TOTAL, rows=ROWS, n_cores=N_CORES):
    """Host-side sharding / layout prep.  Returns list of per-core in_maps."""
    f16 = np.float16
    X = np.asarray(X, np.float32)
    A = np.asarray(A)
    W = np.asarray(W, np.float32)
    a = np.asarray(a, np.float32)

    XT = np.ascontiguousarray(X.T).astype(f16)
    Wcat = np.ascontiguousarray(W.transpose(1, 0, 2).reshape(IN_DIM, HEADS * OUT_DIM))
    a_src, a_dst = a[:, :OUT_DIM], a[:, OUT_DIM:]
    w_src = np.einsum('hdo,ho->hd', W, a_src).astype(np.float32)
    w_dst = np.einsum('hdo,ho->hd', W, a_dst).astype(np.float32)
    W8 = np.concatenate([Wcat, w_dst.T], axis=1).astype(f16)
    W4 = np.ascontiguousarray(w_src.T).astype(f16)
    WSRCB = np.repeat(-w_src.T[:, :, None], P, axis=2).reshape(IN_DIM, HEADS * P)
    WSRCB = np.ascontiguousarray(WSRCB).astype(f16)

    Af = (A > 0).astype(np.float32)
    in_maps = []
    for c in range(n_cores):
        i0 = c * rows
        at = np.ascontiguousarray(Af[i0:i0 + rows, :].T).astype(f16)
        xtown = np.ascontiguousarray(X[i0:i0 + rows, :].T).astype(f16)
        in_maps.append({
            "XT": XT, "XTOWN": xtown, "W8": W8, "W4": W4,
            "WSRCB": WSRCB, "AT": at,
        })
    return in_maps


_CACHED_NC = None


def _get_nc():
    global _CACHED_NC
    if _CACHED_NC is None:
        _CACHED_NC = build_program()
    return _CACHED_NC


def kernel(X, A, W, a, b, _trace=False, _trace_kwargs=None):
    nc = _get_nc()
    in_maps = prep_inputs(X, A, W, a)
    kw = {}
    if _trace:
        kw["trace"] = True
        if _trace_kwargs:
            kw.update(_trace_kwargs)
    res = run_bass_kernel_spmd(nc, in_maps, core_ids=list(range(N_CORES)), **kw)
    out = np.concatenate([r["OUT"] for r in res.results], axis=0)
    out = out + np.asarray(b, np.float32).reshape(1, HEADS * OUT_DIM)
    if _trace:
        return out.astype(np.float32), res
    return out.astype(np.float32)


# revision 25
# speedup vs baseline: 1.1415x; 1.1415x over previous
"""Trainium2 Bass kernel for a 4-head GAT layer (N=4096, D=256, O=64, H=4).

Math (reference):
    feat[h] = X @ W[h]                                  [N, O]
    s[h,i] = feat[h,i] @ a_src[h],  t[h,j] = feat[h,j] @ a_dst[h]
    score[h,i,j] = leaky_relu(s_i + t_j, 0.2), masked by A>0, softmax over j
    out[i, h*O+o] = sum_j attn[h,i,j] feat[h,j,o] + b[h,o]

Key factorization used on-device (avoids 67M-element exp/leaky passes):
    exp(leaky_relu(x)) = max(e^x, e^{0.2x}); with x = s_i + t_j both branches
    factor.  With M2 = A * [x >= 0] and M1 = A - M2:
      numer = e^{0.2 s} * [ (A@(q*f) - M2@(q*f)) + e^{0.8 s} * (M2@(v*f)) ]
    where v = e^t, q = e^{0.2 t}; the common e^{0.2 s} cancels in the softmax
    ratio.  Row sums come from an appended ones-column in the rhs panels.

    Masks are built with DVE fast modes: tensor_scalar is_le (4x mode) for
    m = [-s_i <= t_j] batched [128 x 512], then tensor_tensor mult by the A
    tile (2x mode).  The A-branch accumulates in a prepass fused into head
    pass 0; four single-head passes keep PSUM at 4+4 banks.

Sharding: destination rows are split 512/core across 8 cores; source-side
features (all N) are recomputed per core (cheap).  No collectives.
b is always zero in setup_inputs but is added on the host anyway.
"""

from contextlib import ExitStack

import numpy as np

import concourse.bass as bass
import concourse.tile as tile
import concourse.mybir as mybir
from concourse import bacc
from concourse.bass_utils import run_bass_kernel_spmd

P = 128
IN_DIM = 256
OUT_DIM = 64
HEADS = 4
N_TOTAL = 4096
N_CORES = 8
ROWS = N_TOTAL // N_CORES  # 512

F32 = mybir.dt.float32
F16 = mybir.dt.float16

AL = mybir.AluOpType
AF = mybir.ActivationFunctionType

GRP = 66          # [feat(64) | 1 | 1] per head in the fe panel (even => 4B aligned)
FET_C = 4 * GRP + 4   # 268 cols per n-tile in fet
VQ_C = 2 * GRP    # 132 cols per head in the vfqf panel
PAN_C = 4 * VQ_C  # 528 cols per n-tile in vfqf


def build_program(n_total=N_TOTAL, rows=ROWS, num_devices=N_CORES):
    """Build the per-core SPMD program (same program on all cores; per-core
    data arrives via the input map)."""
    ntiles = n_total // P   # source-node tiles (j)
    nib = rows // P         # destination row blocks per core
    njt = ntiles

    nc = bacc.Bacc("TRN2", target_bir_lowering=False, debug=False,
                   num_devices=num_devices)

    XT = nc.dram_tensor("XT", [IN_DIM, n_total], F16, kind="ExternalInput")
    XTOWN = nc.dram_tensor("XTOWN", [IN_DIM, rows], F16, kind="ExternalInput")
    W8 = nc.dram_tensor("W8", [IN_DIM, 260], F16, kind="ExternalInput")
    W4 = nc.dram_tensor("W4", [IN_DIM, 4], F16, kind="ExternalInput")
    WSRCB = nc.dram_tensor("WSRCB", [IN_DIM, 4 * P], F16, kind="ExternalInput")
    AT = nc.dram_tensor("AT", [n_total, rows], F16, kind="ExternalInput")
    OUT = nc.dram_tensor("OUT", [rows, HEADS * OUT_DIM], F32,
                         kind="ExternalOutput")

    with tile.TileContext(nc) as tc, ExitStack() as ctx:
        big = ctx.enter_context(tc.tile_pool(name="big", bufs=1))

        # ---- Phase 0: load everything ----
        # Small tensors first: the sbc / s_own / feat matmuls need them, and
        # DMA queues drain FIFO -- putting the 6MB of XT/AT ahead of them
        # stalls all compute behind ~25us of bulk DMA.
        xtown_sb = big.tile([P, 2 * rows], F16, tag="xtown")
        for d in range(2):
            nc.sync.dma_start(xtown_sb[:, d * rows:(d + 1) * rows],
                              XTOWN[d * P:(d + 1) * P, :])
        w8_sb = big.tile([P, 2 * 260], F16, tag="w8")
        for d in range(2):
            nc.sync.dma_start(w8_sb[:, d * 260:(d + 1) * 260],
                              W8[d * P:(d + 1) * P, :])
        w4_sb = big.tile([P, 2 * 4], F16, tag="w4")
        for d in range(2):
            nc.sync.dma_start(w4_sb[:, d * 4:(d + 1) * 4],
                              W4[d * P:(d + 1) * P, :])
        wsrcb_sb = big.tile([P, 2 * 4 * P], F16, tag="wsrcb")
        for d in range(2):
            nc.sync.dma_start(wsrcb_sb[:, d * 4 * P:(d + 1) * 4 * P],
                              WSRCB[d * P:(d + 1) * P, :])
        xt_sb = big.tile([P, 2 * n_total], F16, tag="xt")
        nch = 8
        for c in range(nch):
            w = n_total // nch
            for d in range(2):
                nc.sync.dma_start(
                    xt_sb[:, d * n_total + c * w: d * n_total + (c + 1) * w],
                    XT[d * P:(d + 1) * P, c * w:(c + 1) * w])
        at_sb = big.tile([P, njt * rows], F16, tag="at")
        for jt in range(njt):
            nc.sync.dma_start(at_sb[:, jt * rows:(jt + 1) * rows],
                              AT[jt * P:(jt + 1) * P, :])

        # ---- Phase 1: feat + t matmuls; vf/qf panels via ACT scaled copies ----
        # panel layout per n-tile, per head (132 cols):
        #   [vf(64) | v | pad | qf(64) | q | pad]
        t16 = big.tile([P, ntiles * 4], F16, tag="t16")
        t3 = t16[:].rearrange("p (n c) -> p n c", c=4)
        vq = big.tile([P, ntiles * 8], F32, tag="vq")
        vq3 = vq[:].rearrange("p (n c) -> p n c", c=8)
        tpos = big.tile([P, ntiles * 4], F32, tag="tpos")
        tposk = big.tile([P, ntiles * 4], F32, tag="tposk")
        fe = big.tile([P, ntiles * 4 * GRP], F16, tag="fe")
        fe3 = fe[:].rearrange("p (n c) -> p n c", c=4 * GRP)
        vfqf = big.tile([P, ntiles * PAN_C], F16, tag="vfqf")

        # s broadcast rows (NEGATED: host supplies -w_src), grouped per head:
        # sbc[:, h*rows + i] = -s_src[h, i]
        sbc = big.tile([P, 4 * rows], F16, tag="sbc")
        with tc.tile_pool(name="psb", bufs=2, space=bass.MemorySpace.PSUM) as psb:
            for ib in range(nib):
                ps = psb.tile([P, 4 * P], F32, tag="ps_sb")
                for h in range(HEADS):
                    for d in range(2):
                        nc.tensor.matmul(
                            ps[:, h * P:(h + 1) * P],
                            wsrcb_sb[:, d * 4 * P + h * P: d * 4 * P + (h + 1) * P],
                            xtown_sb[:, d * rows + ib * P: d * rows + (ib + 1) * P],
                            start=(d == 0), stop=(d == 1))
                for h in range(HEADS):
                    nc.vector.tensor_copy(
                        sbc[:, h * rows + ib * P: h * rows + (ib + 1) * P],
                        ps[:, h * P:(h + 1) * P])

        s_own = big.tile([P, nib * 4], F32, tag="s_own")
        w_cat = big.tile([P, nib * 4], F32, tag="w_cat")
        with tc.tile_pool(name="pso", bufs=1, space=bass.MemorySpace.PSUM) as pso:
            ps = pso.tile([P, nib * 4], F32, tag="ps_so")
            for ib in range(nib):
                for d in range(2):
                    nc.tensor.matmul(
                        ps[:, ib * 4:(ib + 1) * 4],
                        xtown_sb[:, d * rows + ib * P: d * rows + (ib + 1) * P],
                        w4_sb[:, d * 4:(d + 1) * 4],
                        start=(d == 0), stop=(d == 1))
            nc.vector.tensor_copy(s_own[:], ps[:])
        nc.scalar.activation(w_cat[:], s_own[:], AF.Exp, scale=0.8)

        # ---- feat + t matmuls; panels via DVE 4x tensor_scalar ----
        t16 = big.tile([P, ntiles * 4], F16, tag="t16")
        t3 = t16[:].rearrange("p (n c) -> p n c", c=4)
        vq = big.tile([P, ntiles * 8], F32, tag="vq")
        vq3 = vq[:].rearrange("p (n c) -> p n c", c=8)
        tpos = big.tile([P, ntiles * 4], F32, tag="tpos")
        tposk = big.tile([P, ntiles * 4], F32, tag="tposk")
        fe = big.tile([P, ntiles * 4 * GRP], F16, tag="fe")
        fe3 = fe[:].rearrange("p (n c) -> p n c", c=4 * GRP)
        vfqf = big.tile([P, ntiles * PAN_C], F16, tag="vfqf")

        CHUNK = min(4, ntiles)  # n-tiles per exp chunk
        with tc.tile_pool(name="pfeat", bufs=6, space=bass.MemorySpace.PSUM) as pf:
            for nt0 in range(0, ntiles, CHUNK):
                pss = []
                for nt in range(nt0, nt0 + CHUNK):
                    ps = pf.tile([P, 264], F32, tag="ps")
                    pss.append(ps)
                    for d in range(2):
                        nc.tensor.matmul(
                            ps[:, 0:260],
                            xt_sb[:, d * n_total + nt * P: d * n_total + (nt + 1) * P],
                            w8_sb[:, d * 260:(d + 1) * 260],
                            start=(d == 0), stop=(d == 1))
                    nc.scalar.activation(t3[:, nt, :], ps[:, 256:260], AF.Copy)
                ch = slice(nt0, nt0 + CHUNK)
                nc.scalar.activation(vq3[:, ch, 0:4], t3[:, ch, :], AF.Exp)
                nc.scalar.activation(vq3[:, ch, 4:8], t3[:, ch, :], AF.Exp, scale=0.2)
                nc.vector.tensor_copy(
                    tpos[:, nt0 * 4:(nt0 + CHUNK) * 4], t3[:, ch, :])
                nc.vector.tensor_scalar_mul(
                    tposk[:, nt0 * 4:(nt0 + CHUNK) * 4], t3[:, ch, :], 1.0e4)
                for nt in range(nt0, nt0 + CHUNK):
                    ps = pss[nt - nt0]
                    fe_g = fe3[:, nt, :].rearrange("p (g c) -> p g c", c=GRP)
                    nc.scalar.activation(
                        fe_g[:, :, 0:64],
                        ps[:, 0:256].rearrange("p (g c) -> p g c", c=64),
                        AF.Copy)
                    nc.vector.memset(fe_g[:, :, 64:66], 1.0)
                    pan = vfqf[:, nt * PAN_C:(nt + 1) * PAN_C]
                    for h in range(HEADS - 1):
                        nc.vector.tensor_scalar_mul(
                            pan[:, h * VQ_C: h * VQ_C + GRP],
                            fe_g[:, h, :], vq3[:, nt, h:h + 1])
                        nc.vector.tensor_scalar_mul(
                            pan[:, h * VQ_C + GRP: (h + 1) * VQ_C],
                            fe_g[:, h, :], vq3[:, nt, 4 + h:5 + h])
                    # head 3's panels on the Scalar engine (slack window)
                    h = HEADS - 1
                    nc.scalar.activation(
                        pan[:, h * VQ_C: h * VQ_C + GRP],
                        fe_g[:, h, :], AF.Copy, scale=vq3[:, nt, h:h + 1])
                    nc.scalar.activation(
                        pan[:, h * VQ_C + GRP: (h + 1) * VQ_C],
                        fe_g[:, h, :], AF.Copy, scale=vq3[:, nt, 4 + h:5 + h])

        # ---- Phase 4: fused A-prepass (inside pass 0) + 4 single-head passes ----
        m_pool = ctx.enter_context(tc.tile_pool(name="m", bufs=6))
        m2_pool = ctx.enter_context(tc.tile_pool(name="m2", bufs=8))
        out_sb_pool = ctx.enter_context(tc.tile_pool(name="osb", bufs=4))
        e_pool = ctx.enter_context(tc.tile_pool(name="epi", bufs=6))
        ca_all = []
        for ib in range(nib):
            ca_ib = big.tile([P, 260], F32, tag=f"ca{ib}")
            ca_all.append(ca_ib)
        out_sbs = []
        for ib in range(nib):
            osb = out_sb_pool.tile([P, HEADS * OUT_DIM], F32, tag="outsb")
            out_sbs.append(osb)
        with tc.tile_pool(name="pA", bufs=4, space=bass.MemorySpace.PSUM) as pA, \
             tc.tile_pool(name="pB", bufs=4, space=bass.MemorySpace.PSUM) as pB:
            pa = []
            for ib in range(nib):
                pa_ib = pA.tile([P, 260], F32, tag="pa")
                pa.append(pa_ib)
            for h in range(HEADS):
                pb = []
                for ib in range(nib):
                    pb_ib = pB.tile([P, 130], F32, tag="pb")
                    pb.append(pb_ib)
                for jt in range(njt):
                    a_row = at_sb[:, jt * rows:(jt + 1) * rows]
                    pan = vfqf[:, jt * PAN_C:(jt + 1) * PAN_C]
                    if h == 0:
                        # A-branch: all four heads' qf panels at once
                        qf_all = pan[:].rearrange(
                            "p (g c) -> p g c", c=VQ_C)[:, :, GRP:GRP + 65]
                        for ib in range(nib):
                            nc.tensor.matmul(
                                pa[ib][:],
                                at_sb[:, jt * rows + ib * P: jt * rows + (ib + 1) * P],
                                qf_all,
                                start=(jt == 0), stop=(jt == njt - 1))
                    mb = m_pool.tile([P, rows], F16, tag="mb")
                    if h >= 2:
                        # step via saturated sigmoid on the (idle) Scalar
                        # engine: sigmoid(1e4*(s_i + t_j)); sbc holds -s
                        nc.scalar.activation(
                            mb[:], sbc[:, h * rows:(h + 1) * rows],
                            AF.Sigmoid, scale=-1.0e4,
                            bias=tposk[:, jt * 4 + h: jt * 4 + h + 1])
                    else:
                        # m = [s_i + t_j >= 0] = [-s_i <= t_j]
                        nc.vector.tensor_scalar(
                            mb[:], sbc[:, h * rows:(h + 1) * rows],
                            tpos[:, jt * 4 + h: jt * 4 + h + 1],
                            None, AL.is_le)
                    m2 = m2_pool.tile([P, rows], F16, tag="m2b")
                    nc.vector.tensor_tensor(m2[:], mb[:], a_row, AL.mult)
                    vfqf_h = pan[:, h * VQ_C: h * VQ_C + 2 * GRP].rearrange(
                        "p (g c) -> p g c", c=GRP)[:, :, 0:65]
                    for ib in range(nib):
                        nc.tensor.matmul(
                            pb[ib][:],
                            m2[:, ib * P:(ib + 1) * P],
                            vfqf_h,
                            start=(jt == 0), stop=(jt == njt - 1))
                if h == 0:
                    for ib in range(nib):
                        nc.scalar.activation(ca_all[ib][:], pa[ib][:], AF.Copy)
                # epilogue for head h
                for ib in range(nib):
                    dh = e_pool.tile([P, 65], F32, tag="dh")
                    nc.vector.tensor_sub(
                        dh[:], ca_all[ib][:, h * 65:(h + 1) * 65],
                        pb[ib][:, 65:130])
                    zh = e_pool.tile([P, 65], F32, tag="zh")
                    nc.vector.scalar_tensor_tensor(
                        zh[:], pb[ib][:, 0:65],
                        w_cat[:, ib * 4 + h: ib * 4 + h + 1],
                        dh[:], AL.mult, AL.add)
                    rc = e_pool.tile([P, 1], F32, tag="rc")
                    nc.vector.reciprocal(rc[:], zh[:, 64:65])
                    nc.vector.tensor_scalar_mul(
                        out_sbs[ib][:, h * OUT_DIM:(h + 1) * OUT_DIM],
                        zh[:, 0:OUT_DIM], rc[:])
        for ib in range(nib):
            nc.sync.dma_start(OUT[ib * P:(ib + 1) * P, :], out_sbs[ib][:])

    nc.compile()
    return nc


def prep_inputs(X, A, W, a, n_total=N_TOTAL, rows=ROWS, n_cores=N_CORES):
    """Host-side sharding / layout prep.  Returns list of per-core in_maps."""
    f16 = np.float16
    X = np.asarray(X, np.float32)
    A = np.asarray(A)
    W = np.asarray(W, np.float32)
    a = np.asarray(a, np.float32)

    XT = np.ascontiguousarray(X.T).astype(f16)
    Wcat = np.ascontiguousarray(W.transpose(1, 0, 2).reshape(IN_DIM, HEADS * OUT_DIM))
    a_src, a_dst = a[:, :OUT_DIM], a[:, OUT_DIM:]
    w_src = np.einsum('hdo,ho->hd', W, a_src).astype(np.float32)
    w_dst = np.einsum('hdo,ho->hd', W, a_dst).astype(np.float32)
    W8 = np.concatenate([Wcat, w_dst.T], axis=1).astype(f16)
    W4 = np.ascontiguousarray(w_src.T).astype(f16)
    WSRCB = np.repeat(-w_src.T[:, :, None], P, axis=2).reshape(IN_DIM, HEADS * P)
    WSRCB = np.ascontiguousarray(WSRCB).astype(f16)

    Af = (A > 0).astype(np.float32)
    in_maps = []
    for c in range(n_cores):
        i0 = c * rows
        at = np.ascontiguousarray(Af[i0:i0 + rows, :].T).astype(f16)
        xtown = np.ascontiguousarray(X[i0:i0 + rows, :].T).astype(f16)
        in_maps.append({
            "XT": XT, "XTOWN": xtown, "W8": W8, "W4": W4,
            "WSRCB": WSRCB, "AT": at,
        })
    return in_maps


_CACHED_NC = None


def _get_nc():
    global _CACHED_NC
    if _CACHED_NC is None:
        _CACHED_NC = build_program()
    return _CACHED_NC


def kernel(X, A, W, a, b, _trace=False, _trace_kwargs=None):
    nc = _get_nc()
    in_maps = prep_inputs(X, A, W, a)
    kw = {}
    if _trace:
        kw["trace"] = True
        if _trace_kwargs:
            kw.update(_trace_kwargs)
    res = run_bass_kernel_spmd(nc, in_maps, core_ids=list(range(N_CORES)), **kw)
    out = np.concatenate([r["OUT"] for r in res.results], axis=0)
    out = out + np.asarray(b, np.float32).reshape(1, HEADS * OUT_DIM)
    if _trace:
        return out.astype(np.float32), res
    return out.astype(np.float32)


# revision 26
# speedup vs baseline: 1.1490x; 1.0065x over previous
"""Trainium2 Bass kernel for a 4-head GAT layer (N=4096, D=256, O=64, H=4).

Math (reference):
    feat[h] = X @ W[h]                                  [N, O]
    s[h,i] = feat[h,i] @ a_src[h],  t[h,j] = feat[h,j] @ a_dst[h]
    score[h,i,j] = leaky_relu(s_i + t_j, 0.2), masked by A>0, softmax over j
    out[i, h*O+o] = sum_j attn[h,i,j] feat[h,j,o] + b[h,o]

Key factorization used on-device (avoids 67M-element exp/leaky passes):
    exp(leaky_relu(x)) = max(e^x, e^{0.2x}); with x = s_i + t_j both branches
    factor.  With M2 = A * [x >= 0] and M1 = A - M2:
      numer = e^{0.2 s} * [ (A@(q*f) - M2@(q*f)) + e^{0.8 s} * (M2@(v*f)) ]
    where v = e^t, q = e^{0.2 t}; the common e^{0.2 s} cancels in the softmax
    ratio.  Row sums come from an appended ones-column in the rhs panels.

    Masks are built with DVE fast modes: tensor_scalar is_le (4x mode) for
    m = [-s_i <= t_j] batched [128 x 512], then tensor_tensor mult by the A
    tile (2x mode).  The A-branch accumulates in a prepass fused into head
    pass 0; four single-head passes keep PSUM at 4+4 banks.

Sharding: destination rows are split 512/core across 8 cores; source-side
features (all N) are recomputed per core (cheap).  No collectives.
b is always zero in setup_inputs but is added on the host anyway.
"""

from contextlib import ExitStack

import numpy as np

import concourse.bass as bass
import concourse.tile as tile
import concourse.mybir as mybir
from concourse import bacc
from concourse.bass_utils import run_bass_kernel_spmd

P = 128
IN_DIM = 256
OUT_DIM = 64
HEADS = 4
N_TOTAL = 4096
N_CORES = 8
ROWS = N_TOTAL // N_CORES  # 512

F32 = mybir.dt.float32
F16 = mybir.dt.float16

AL = mybir.AluOpType
AF = mybir.ActivationFunctionType

GRP = 66          # [feat(64) | 1 | 1] per head in the fe panel (even => 4B aligned)
FET_C = 4 * GRP + 4   # 268 cols per n-tile in fet
VQ_C = 2 * GRP    # 132 cols per head in the vfqf panel
PAN_C = 4 * VQ_C  # 528 cols per n-tile in vfqf


def build_program(n_total=N_TOTAL, rows=ROWS, num_devices=N_CORES):
    """Build the per-core SPMD program (same program on all cores; per-core
    data arrives via the input map)."""
    ntiles = n_total // P   # source-node tiles (j)
    nib = rows // P         # destination row blocks per core
    njt = ntiles

    nc = bacc.Bacc("TRN2", target_bir_lowering=False, debug=False,
                   num_devices=num_devices)

    XT = nc.dram_tensor("XT", [IN_DIM, n_total], F16, kind="ExternalInput")
    XTOWN = nc.dram_tensor("XTOWN", [IN_DIM, rows], F16, kind="ExternalInput")
    W8 = nc.dram_tensor("W8", [IN_DIM, 260], F16, kind="ExternalInput")
    W4 = nc.dram_tensor("W4", [IN_DIM, 4], F16, kind="ExternalInput")
    WSRCB = nc.dram_tensor("WSRCB", [IN_DIM, 4 * P], F16, kind="ExternalInput")
    AT = nc.dram_tensor("AT", [n_total, rows], F16, kind="ExternalInput")
    OUT = nc.dram_tensor("OUT", [rows, HEADS * OUT_DIM], F32,
                         kind="ExternalOutput")

    with tile.TileContext(nc) as tc, ExitStack() as ctx:
        big = ctx.enter_context(tc.tile_pool(name="big", bufs=1))

        # ---- Phase 0: load everything ----
        # Small tensors first: the sbc / s_own / feat matmuls need them, and
        # DMA queues drain FIFO -- putting the 6MB of XT/AT ahead of them
        # stalls all compute behind ~25us of bulk DMA.
        xtown_sb = big.tile([P, 2 * rows], F16, tag="xtown")
        for d in range(2):
            nc.sync.dma_start(xtown_sb[:, d * rows:(d + 1) * rows],
                              XTOWN[d * P:(d + 1) * P, :])
        w8_sb = big.tile([P, 2 * 260], F16, tag="w8")
        for d in range(2):
            nc.sync.dma_start(w8_sb[:, d * 260:(d + 1) * 260],
                              W8[d * P:(d + 1) * P, :])
        w4_sb = big.tile([P, 2 * 4], F16, tag="w4")
        for d in range(2):
            nc.sync.dma_start(w4_sb[:, d * 4:(d + 1) * 4],
                              W4[d * P:(d + 1) * P, :])
        wsrcb_sb = big.tile([P, 2 * 4 * P], F16, tag="wsrcb")
        for d in range(2):
            nc.sync.dma_start(wsrcb_sb[:, d * 4 * P:(d + 1) * 4 * P],
                              WSRCB[d * P:(d + 1) * P, :])
        xt_sb = big.tile([P, 2 * n_total], F16, tag="xt")
        nch = 8
        for c in range(nch):
            w = n_total // nch
            for d in range(2):
                nc.sync.dma_start(
                    xt_sb[:, d * n_total + c * w: d * n_total + (c + 1) * w],
                    XT[d * P:(d + 1) * P, c * w:(c + 1) * w])
        at_sb = big.tile([P, njt * rows], F16, tag="at")
        for jt in range(njt):
            nc.sync.dma_start(at_sb[:, jt * rows:(jt + 1) * rows],
                              AT[jt * P:(jt + 1) * P, :])

        # ---- Phase 1: feat + t matmuls; vf/qf panels via ACT scaled copies ----
        # panel layout per n-tile, per head (132 cols):
        #   [vf(64) | v | pad | qf(64) | q | pad]
        t16 = big.tile([P, ntiles * 4], F16, tag="t16")
        t3 = t16[:].rearrange("p (n c) -> p n c", c=4)
        vq = big.tile([P, ntiles * 8], F32, tag="vq")
        vq3 = vq[:].rearrange("p (n c) -> p n c", c=8)
        tpos = big.tile([P, ntiles * 4], F32, tag="tpos")
        tposk = big.tile([P, ntiles * 4], F32, tag="tposk")
        fe = big.tile([P, ntiles * 4 * GRP], F16, tag="fe")
        fe3 = fe[:].rearrange("p (n c) -> p n c", c=4 * GRP)
        vfqf = big.tile([P, ntiles * PAN_C], F16, tag="vfqf")

        # s broadcast rows (NEGATED: host supplies -w_src), grouped per head:
        # sbc[:, h*rows + i] = -s_src[h, i]
        sbc = big.tile([P, 4 * rows], F16, tag="sbc")
        with tc.tile_pool(name="psb", bufs=2, space=bass.MemorySpace.PSUM) as psb:
            for ib in range(nib):
                ps = psb.tile([P, 4 * P], F32, tag="ps_sb")
                for h in range(HEADS):
                    for d in range(2):
                        nc.tensor.matmul(
                            ps[:, h * P:(h + 1) * P],
                            wsrcb_sb[:, d * 4 * P + h * P: d * 4 * P + (h + 1) * P],
                            xtown_sb[:, d * rows + ib * P: d * rows + (ib + 1) * P],
                            start=(d == 0), stop=(d == 1))
                for h in range(HEADS):
                    nc.vector.tensor_copy(
                        sbc[:, h * rows + ib * P: h * rows + (ib + 1) * P],
                        ps[:, h * P:(h + 1) * P])

        s_own = big.tile([P, nib * 4], F32, tag="s_own")
        w_cat = big.tile([P, nib * 4], F32, tag="w_cat")
        with tc.tile_pool(name="pso", bufs=1, space=bass.MemorySpace.PSUM) as pso:
            ps = pso.tile([P, nib * 4], F32, tag="ps_so")
            for ib in range(nib):
                for d in range(2):
                    nc.tensor.matmul(
                        ps[:, ib * 4:(ib + 1) * 4],
                        xtown_sb[:, d * rows + ib * P: d * rows + (ib + 1) * P],
                        w4_sb[:, d * 4:(d + 1) * 4],
                        start=(d == 0), stop=(d == 1))
            nc.vector.tensor_copy(s_own[:], ps[:])
        nc.scalar.activation(w_cat[:], s_own[:], AF.Exp, scale=0.8)

        # ---- feat + t matmuls; panels via DVE 4x tensor_scalar ----
        t16 = big.tile([P, ntiles * 4], F16, tag="t16")
        t3 = t16[:].rearrange("p (n c) -> p n c", c=4)
        vq = big.tile([P, ntiles * 8], F32, tag="vq")
        vq3 = vq[:].rearrange("p (n c) -> p n c", c=8)
        tpos = big.tile([P, ntiles * 4], F32, tag="tpos")
        tposk = big.tile([P, ntiles * 4], F32, tag="tposk")
        fe = big.tile([P, ntiles * 4 * GRP], F16, tag="fe")
        fe3 = fe[:].rearrange("p (n c) -> p n c", c=4 * GRP)
        vfqf = big.tile([P, ntiles * PAN_C], F16, tag="vfqf")

        CHUNK = min(4, ntiles)  # n-tiles per exp chunk
        with tc.tile_pool(name="pfeat", bufs=6, space=bass.MemorySpace.PSUM) as pf:
            for nt0 in range(0, ntiles, CHUNK):
                pss = []
                for nt in range(nt0, nt0 + CHUNK):
                    ps = pf.tile([P, 264], F32, tag="ps")
                    pss.append(ps)
                    for d in range(2):
                        nc.tensor.matmul(
                            ps[:, 0:260],
                            xt_sb[:, d * n_total + nt * P: d * n_total + (nt + 1) * P],
                            w8_sb[:, d * 260:(d + 1) * 260],
                            start=(d == 0), stop=(d == 1))
                    nc.scalar.activation(t3[:, nt, :], ps[:, 256:260], AF.Copy)
                ch = slice(nt0, nt0 + CHUNK)
                nc.scalar.activation(vq3[:, ch, 0:4], t3[:, ch, :], AF.Exp)
                nc.scalar.activation(vq3[:, ch, 4:8], t3[:, ch, :], AF.Exp, scale=0.2)
                nc.vector.tensor_copy(
                    tpos[:, nt0 * 4:(nt0 + CHUNK) * 4], t3[:, ch, :])
                nc.vector.tensor_scalar_mul(
                    tposk[:, nt0 * 4:(nt0 + CHUNK) * 4], t3[:, ch, :], 1.0e4)
                for nt in range(nt0, nt0 + CHUNK):
                    ps = pss[nt - nt0]
                    fe_g = fe3[:, nt, :].rearrange("p (g c) -> p g c", c=GRP)
                    nc.scalar.activation(
                        fe_g[:, :, 0:64],
                        ps[:, 0:256].rearrange("p (g c) -> p g c", c=64),
                        AF.Copy)
                    nc.vector.memset(fe_g[:, :, 64:66], 1.0)
                    pan = vfqf[:, nt * PAN_C:(nt + 1) * PAN_C]
                    for h in range(HEADS - 1):
                        nc.vector.tensor_scalar_mul(
                            pan[:, h * VQ_C: h * VQ_C + GRP],
                            fe_g[:, h, :], vq3[:, nt, h:h + 1])
                        nc.vector.tensor_scalar_mul(
                            pan[:, h * VQ_C + GRP: (h + 1) * VQ_C],
                            fe_g[:, h, :], vq3[:, nt, 4 + h:5 + h])
                    # head 3's panels on the Scalar engine (slack window)
                    h = HEADS - 1
                    nc.scalar.activation(
                        pan[:, h * VQ_C: h * VQ_C + GRP],
                        fe_g[:, h, :], AF.Copy, scale=vq3[:, nt, h:h + 1])
                    nc.scalar.activation(
                        pan[:, h * VQ_C + GRP: (h + 1) * VQ_C],
                        fe_g[:, h, :], AF.Copy, scale=vq3[:, nt, 4 + h:5 + h])

        # ---- Phase 4: fused A-prepass (inside pass 0) + 4 single-head passes ----
        m_pool = ctx.enter_context(tc.tile_pool(name="m", bufs=6))
        m2_pool = ctx.enter_context(tc.tile_pool(name="m2", bufs=8))
        out_sb_pool = ctx.enter_context(tc.tile_pool(name="osb", bufs=4))
        e_pool = ctx.enter_context(tc.tile_pool(name="epi", bufs=6))
        ca_all = []
        for ib in range(nib):
            ca_ib = big.tile([P, 260], F32, tag=f"ca{ib}")
            ca_all.append(ca_ib)
        out_sbs = []
        for ib in range(nib):
            osb = out_sb_pool.tile([P, HEADS * OUT_DIM], F32, tag="outsb")
            out_sbs.append(osb)
        with tc.tile_pool(name="pA", bufs=4, space=bass.MemorySpace.PSUM) as pA, \
             tc.tile_pool(name="pB", bufs=4, space=bass.MemorySpace.PSUM) as pB:
            pa = []
            for ib in range(nib):
                pa_ib = pA.tile([P, 260], F32, tag="pa")
                pa.append(pa_ib)
            for h in (0, 2, 1, 3):   # alternate DVE-compare / ACT-sigmoid passes
                pb = []
                for ib in range(nib):
                    pb_ib = pB.tile([P, 130], F32, tag="pb")
                    pb.append(pb_ib)
                for jt in range(njt):
                    a_row = at_sb[:, jt * rows:(jt + 1) * rows]
                    pan = vfqf[:, jt * PAN_C:(jt + 1) * PAN_C]
                    if h == 0:
                        # A-branch: all four heads' qf panels at once
                        qf_all = pan[:].rearrange(
                            "p (g c) -> p g c", c=VQ_C)[:, :, GRP:GRP + 65]
                        for ib in range(nib):
                            nc.tensor.matmul(
                                pa[ib][:],
                                at_sb[:, jt * rows + ib * P: jt * rows + (ib + 1) * P],
                                qf_all,
                                start=(jt == 0), stop=(jt == njt - 1))
                    mb = m_pool.tile([P, rows], F16, tag="mb")
                    if h >= 2:
                        # step via saturated sigmoid on the (idle) Scalar
                        # engine: sigmoid(1e4*(s_i + t_j)); sbc holds -s
                        nc.scalar.activation(
                            mb[:], sbc[:, h * rows:(h + 1) * rows],
                            AF.Sigmoid, scale=-1.0e4,
                            bias=tposk[:, jt * 4 + h: jt * 4 + h + 1])
                    else:
                        # m = [s_i + t_j >= 0] = [-s_i <= t_j]
                        nc.vector.tensor_scalar(
                            mb[:], sbc[:, h * rows:(h + 1) * rows],
                            tpos[:, jt * 4 + h: jt * 4 + h + 1],
                            None, AL.is_le)
                    m2 = m2_pool.tile([P, rows], F16, tag="m2b")
                    nc.vector.tensor_tensor(m2[:], mb[:], a_row, AL.mult)
                    vfqf_h = pan[:, h * VQ_C: h * VQ_C + 2 * GRP].rearrange(
                        "p (g c) -> p g c", c=GRP)[:, :, 0:65]
                    for ib in range(nib):
                        nc.tensor.matmul(
                            pb[ib][:],
                            m2[:, ib * P:(ib + 1) * P],
                            vfqf_h,
                            start=(jt == 0), stop=(jt == njt - 1))
                if h == 0:
                    for ib in range(nib):
                        nc.scalar.activation(ca_all[ib][:], pa[ib][:], AF.Copy)
                # epilogue for head h
                for ib in range(nib):
                    dh = e_pool.tile([P, 65], F32, tag="dh")
                    nc.vector.tensor_sub(
                        dh[:], ca_all[ib][:, h * 65:(h + 1) * 65],
                        pb[ib][:, 65:130])
                    zh = e_pool.tile([P, 65], F32, tag="zh")
                    nc.vector.scalar_tensor_tensor(
                        zh[:], pb[ib][:, 0:65],
                        w_cat[:, ib * 4 + h: ib * 4 + h + 1],
                        dh[:], AL.mult, AL.add)
                    rc = e_pool.tile([P, 1], F32, tag="rc")
                    nc.vector.reciprocal(rc[:], zh[:, 64:65])
                    nc.vector.tensor_scalar_mul(
                        out_sbs[ib][:, h * OUT_DIM:(h + 1) * OUT_DIM],
                        zh[:, 0:OUT_DIM], rc[:])
        for ib in range(nib):
            nc.sync.dma_start(OUT[ib * P:(ib + 1) * P, :], out_sbs[ib][:])

    nc.compile()
    return nc


def prep_inputs(X, A, W, a, n_total=N_TOTAL, rows=ROWS, n_cores=N_CORES):
    """Host-side sharding / layout prep.  Returns list of per-core in_maps."""
    f16 = np.float16
    X = np.asarray(X, np.float32)
    A = np.asarray(A)
    W = np.asarray(W, np.float32)
    a = np.asarray(a, np.float32)

    XT = np.ascontiguousarray(X.T).astype(f16)
    Wcat = np.ascontiguousarray(W.transpose(1, 0, 2).reshape(IN_DIM, HEADS * OUT_DIM))
    a_src, a_dst = a[:, :OUT_DIM], a[:, OUT_DIM:]
    w_src = np.einsum('hdo,ho->hd', W, a_src).astype(np.float32)
    w_dst = np.einsum('hdo,ho->hd', W, a_dst).astype(np.float32)
    W8 = np.concatenate([Wcat, w_dst.T], axis=1).astype(f16)
    W4 = np.ascontiguousarray(w_src.T).astype(f16)
    WSRCB = np.repeat(-w_src.T[:, :, None], P, axis=2).reshape(IN_DIM, HEADS * P)
    WSRCB = np.ascontiguousarray(WSRCB).astype(f16)

    Af = (A > 0).astype(np.float32)
    in_maps = []
    for c in range(n_cores):
        i0 = c * rows
        at = np.ascontiguousarray(Af[i0:i0 + rows, :].T).astype(f16)
        xtown = np.ascontiguousarray(X[i0:i0 + rows, :].T).astype(f16)
        in_maps.append({
            "XT": XT, "XTOWN": xtown, "W8": W8, "W4": W4,
            "WSRCB": WSRCB, "AT": at,
        })
    return in_maps


_CACHED_NC = None


def _get_nc():
    global _CACHED_NC
    if _CACHED_NC is None:
        _CACHED_NC = build_program()
    return _CACHED_NC


def kernel(X, A, W, a, b, _trace=False, _trace_kwargs=None):
    nc = _get_nc()
    in_maps = prep_inputs(X, A, W, a)
    kw = {}
    if _trace:
        kw["trace"] = True
        if _trace_kwargs:
            kw.update(_trace_kwargs)
    res = run_bass_kernel_spmd(nc, in_maps, core_ids=list(range(N_CORES)), **kw)
    out = np.concatenate([r["OUT"] for r in res.results], axis=0)
    out = out + np.asarray(b, np.float32).reshape(1, HEADS * OUT_DIM)
    if _trace:
        return out.astype(np.float32), res
    return out.astype(np.float32)
